# revision 2
# baseline (speedup 1.0000x reference)
"""Self-contained TRN2 Bass kernel: single-head encoder self-attention.

kernel(**inputs) takes the FULL unsharded inputs (as produced by the
problem's setup_inputs) and returns the full [2, 4096, 128] fp32 output:

    out = softmax((X Wq / sqrt(128)) (X Wk)^T, axis=keys) @ (X Wv)

Note the reference adds its mask along the *query* axis broadcast over
keys, which is a no-op under softmax, so the masks (and the unused
encoder_output_embedding / target mask) do not affect the output.

Sharding: sequence-parallel over queries. Core c handles batch c//4,
query rows (c%4)*1024 ... +1024; keys/values (from the full batch
sequence) are recomputed per core from X and the replicated 128x128
weights.

Per-core program (measured-HW-aware design; fp32r matmuls keep a FIXED
stationary operand — reloading an fp32r stationary costs ~1.4us on TRN2
while bf16 reloads are ~free):
  setup: PE-transpose X, XQ to get X^T/XQ^T (fp32), project K^T/Q^T
         (fp32r, stationary = weights) and V^T (stationary = Wv), cast
         V to bf16 [t, dv] chunks via bf16 PE transposes.
  per 128-query sub-block:
    pass1: S chunks (stationary = Q^T slice, moving = K^T, fp32r) ->
           DVE negated max-reduce -> -m [128,1].
    pass2: same matmuls again; ACT Exp(bias=-m, accum_out -> l) writes
           P as bf16; DVE reduce+reciprocal -> 1/l.
  per 512-query group: per key chunk, 4x bf16 PE transposes of P ->
    P^T, one DVE copy to SBUF, PV accumulates OUT^T = V^T P^T in PSUM
    (stationary = bf16 V chunk); PE transposes OUT^T back (fp32), ACT
    scales rows by 1/l, DMA out.
"""
import contextlib

import numpy as np

import concourse.tile as tile
from concourse import bacc, mybir
from concourse.bass_utils import run_bass_kernel_spmd

F32 = mybir.dt.float32
F32R = mybir.dt.float32r
BF16 = mybir.dt.float16  # fp16: 10-bit mantissa, fits probs/V
AX = mybir.AxisListType
ALU = mybir.AluOpType
ACTF = mybir.ActivationFunctionType

D = 128
B_SZ = 2
S_SRC = 4096
N_CORES = 8
N_ROWS = (B_SZ * S_SRC) // N_CORES  # 1024 query rows per core

_NC_CACHE = {}


def _make_identity(nc, ap):
    nc.gpsimd.memset(ap, 0.0)
    nc.gpsimd.affine_select(
        out=ap, in_=ap, compare_op=ALU.not_equal, fill=1.0, base=0,
        pattern=[[-1, ap.shape[1]]], channel_multiplier=1)


def _build_attn(n_tok=S_SRC, n_rows=N_ROWS, n_cores=N_CORES, m_repeat=None):
    kch = n_tok // 512      # 512-key chunks (pass1/pass2 matmuls)
    tch = n_tok // 128      # 128-key chunks (transpose/PV)
    nsb = n_rows // 128     # 128-query sub-blocks
    spg = min(4, nsb)       # sub-blocks per PV group

    nc = bacc.Bacc("TRN2", target_bir_lowering=False, debug=False,
                   num_devices=n_cores)
    x_d = nc.dram_tensor("x", [n_tok, D], F32, kind="ExternalInput")
    xq_d = nc.dram_tensor("xq", [n_rows, D], F32, kind="ExternalInput")
    wq_d = nc.dram_tensor("wq", [D, D], F32, kind="ExternalInput")
    wk_d = nc.dram_tensor("wk", [D, D], F32, kind="ExternalInput")
    wv_d = nc.dram_tensor("wv", [D, D], F32, kind="ExternalInput")
    out_d = nc.dram_tensor("out", [n_rows, D], F32, kind="ExternalOutput")

    with tile.TileContext(nc) as tc:
        with tc.tile_pool(name="const", bufs=1) as constp, \
             tc.tile_pool(name="big", bufs=1) as bigp, \
             tc.tile_pool(name="xin", bufs=3) as xinp, \
             tc.tile_pool(name="pbuf", bufs=6) as pbufp, \
             tc.tile_pool(name="ptsb", bufs=4) as ptsbp, \
             tc.tile_pool(name="sm", bufs=3) as smp, \
             tc.tile_pool(name="lrp", bufs=10) as lrp, \
             tc.tile_pool(name="osb", bufs=2) as osbp, \
             tc.tile_pool(name="ps1", bufs=2, space="PSUM") as ps1, \
             tc.tile_pool(name="ps2", bufs=2, space="PSUM") as ps2, \
             tc.tile_pool(name="pspt", bufs=2, space="PSUM") as pspt, \
             tc.tile_pool(name="pspv", bufs=1, space="PSUM") as pspv, \
             tc.tile_pool(name="psm", bufs=1, space="PSUM") as psm:

            wq_t = constp.tile([D, D], F32R, tag="wq")
            wk_t = constp.tile([D, D], F32R, tag="wk")
            wv_t = constp.tile([D, D], F32R, tag="wv")
            for w_d, w_t, wtag in [(wq_d, wq_t, "wqf"), (wk_d, wk_t, "wkf"),
                                   (wv_d, wv_t, "wvf")]:
                w_f = constp.tile([D, D], F32, tag=wtag)
                nc.sync.dma_start(out=w_f[:], in_=w_d.ap())
                nc.scalar.copy(w_t[:], w_f[:])
            id_f = constp.tile([D, D], F32, tag="id_f")
            _make_identity(nc, id_f[:])
            id_b = constp.tile([D, D], BF16, tag="id_b")
            nc.scalar.copy(id_b[:], id_f[:])

            rep_ctx = tc.For_i(0, m_repeat, 1) if m_repeat else \
                contextlib.nullcontext()
            rep_ctx.__enter__()

            xT = bigp.tile([D, n_tok], F32R, tag="xT")
            xqT = bigp.tile([D, n_rows], F32R, tag="xqT")
            kT = bigp.tile([D, n_tok], F32R, tag="kT")
            qT = bigp.tile([D, n_rows], F32R, tag="qT")
            vTb = bigp.tile([D, n_tok], BF16, tag="vTb")    # V^T bf16 [dv, t]
            v_b = bigp.tile([D, tch, D], BF16, tag="v_b")   # V bf16 [t, j, dv]

            def setup_copy(i, out_ap, in_ap):
                if i % 2 == 0:
                    nc.scalar.copy(out_ap, in_ap)
                else:
                    nc.vector.tensor_copy(out_ap, in_ap)

            def load_transpose(src_d, n, dstT):
                for c0 in range(0, n, 512):
                    w = min(512, n - c0)
                    slab = xinp.tile([D, 512], F32, tag="xslab")
                    nc.sync.dma_start(
                        out=slab[:, :w].rearrange("p (j d) -> p j d", d=D),
                        in_=src_d.ap()[c0:c0 + w, :]
                            .rearrange("(j p) d -> p j d", p=D))
                    pst = psm.tile([D, 512], F32, tag="psm")
                    for jj in range(w // D):
                        nc.tensor.transpose(pst[:, jj * D:(jj + 1) * D],
                                            slab[:, jj * D:(jj + 1) * D], id_f[:])
                    setup_copy(c0 // 512, dstT[:, c0:c0 + w], pst[:, :w])

            load_transpose(xq_d, n_rows, xqT)
            for c0 in range(0, n_rows, 512):
                w = min(512, n_rows - c0)
                pq = psm.tile([D, 512], F32, tag="psm")
                nc.tensor.matmul(pq[:, :w], lhsT=wq_t[:], rhs=xqT[:, c0:c0 + w],
                                 start=True, stop=True)
                setup_copy(c0 // 512, qT[:, c0:c0 + w], pq[:, :w])
            load_transpose(x_d, n_tok, xT)
            for c0 in range(0, n_tok, 512):
                pk = psm.tile([D, 512], F32, tag="psm")
                nc.tensor.matmul(pk[:], lhsT=wk_t[:], rhs=xT[:, c0:c0 + 512],
                                 start=True, stop=True)
                setup_copy(c0 // 512, kT[:, c0:c0 + 512], pk[:])

            def v_proj():
                for c0 in range(0, n_tok, 512):
                    pv = psm.tile([D, 512], F32, tag="psm")
                    nc.tensor.matmul(pv[:], lhsT=wv_t[:], rhs=xT[:, c0:c0 + 512],
                                     start=True, stop=True)
                    setup_copy(c0 // 512, vTb[:, c0:c0 + 512], pv[:])
                for c0 in range(0, n_tok, 512):
                    pvt = pspt.tile([D, 4, D], BF16, tag="pspt")
                    for jj in range(4):
                        j = c0 // D + jj
                        nc.tensor.transpose(
                            pvt[:, jj:jj + 1, :].rearrange("p a b -> p (a b)"),
                            vTb[:, j * D:(j + 1) * D], id_b[:])
                    nc.vector.tensor_copy(
                        v_b[:, c0 // D:c0 // D + 4, :]
                            .rearrange("p a b -> p (a b)"),
                        pvt[:].rearrange("p a b -> p (a b)"))

            negms, lrecs, p_sb = {}, {}, {}

            def pass1(s):
                r0 = s * 128
                m8 = smp.tile([128, kch], F32, tag="m8")
                for cc in range(kch):
                    pa = ps1.tile([128, 512], F32, tag="ps1")
                    nc.tensor.matmul(pa[:], lhsT=qT[:, r0:r0 + 128],
                                     rhs=kT[:, cc * 512:(cc + 1) * 512],
                                     start=True, stop=True)
                    nc.vector.tensor_reduce(m8[:, cc:cc + 1], pa[:],
                                            axis=AX.X, op=ALU.max, negate=True)
                negm = smp.tile([128, 1], F32, tag="negm")
                nc.vector.tensor_reduce(negm[:], m8[:], axis=AX.X, op=ALU.min)
                negms[s] = negm

            def pass2(s):
                r0 = s * 128
                ps = pbufp.tile([128, n_tok], BF16, tag="p_s")
                p_sb[s] = ps
                l8 = smp.tile([128, kch], F32, tag="l8")
                for cc in range(kch):
                    pa = ps2.tile([128, 512], F32, tag="ps2")
                    nc.tensor.matmul(pa[:], lhsT=qT[:, r0:r0 + 128],
                                     rhs=kT[:, cc * 512:(cc + 1) * 512],
                                     start=True, stop=True)
                    nc.scalar.activation(ps[:, cc * 512:(cc + 1) * 512], pa[:],
                                         ACTF.Exp, bias=negms[s][:, 0:1],
                                         accum_out=l8[:, cc:cc + 1])
                lsum = smp.tile([128, 1], F32, tag="lsum")
                nc.vector.tensor_reduce(lsum[:], l8[:], axis=AX.X, op=ALU.add)
                lrec = lrp.tile([128, 1], F32, tag="lrec")
                nc.vector.reciprocal(lrec[:], lsum[:])
                lrecs[s] = lrec

            def group(g):
                subs = list(range(g * spg, g * spg + spg))
                rw = spg * 128
                ppv = pspv.tile([D, rw], F32, tag="pspv")
                for j0 in range(0, tch, 2):
                    ptp = pspt.tile([D, 2, spg, D], BF16, tag="pspt")
                    for h in range(2):
                        j = j0 + h
                        for si, s in enumerate(subs):
                            nc.tensor.transpose(
                                ptp[:, h, si:si + 1, :]
                                    .rearrange("p a b -> p (a b)"),
                                p_sb[s][:, j * D:(j + 1) * D], id_b[:])
                    ptsb = ptsbp.tile([128, 2, rw], BF16, tag="ptsb")
                    nc.vector.tensor_copy(
                        ptsb[:].rearrange("p a b -> p (a b)"),
                        ptp[:].rearrange("p a b c -> p (a b c)"))
                    for h in range(2):
                        j = j0 + h
                        nc.tensor.matmul(ppv[:], lhsT=v_b[:, j, :],
                                         rhs=ptsb[:, h, :],
                                         start=(j == 0), stop=(j == tch - 1))
                osb = osbp.tile([D, rw], F32, tag="osb")
                nc.scalar.copy(osb[:], ppv[:])
                ofin = osbp.tile([128, rw], F32, tag="ofin")
                for si, s in enumerate(subs):
                    pob = psm.tile([128, 512], F32, tag="psm")
                    nc.tensor.transpose(pob[:, 0:D],
                                        osb[:, si * D:(si + 1) * D], id_f[:])
                    nc.scalar.mul(ofin[:, si * D:(si + 1) * D], pob[:, 0:D],
                                  lrecs[s][:, 0:1])
                nc.sync.dma_start(
                    out=out_d.ap()[g * rw:(g + 1) * rw, :]
                        .rearrange("(s p) d -> p s d", p=D),
                    in_=ofin[:].rearrange("p (s d) -> p s d", d=D))

            pass1(0)
            v_proj()
            pass2(0)
            for s in range(1, nsb):
                pass1(s)
                pass2(s)
                if s % spg == 0 and s >= spg:
                    group(s // spg - 1)
            group(nsb // spg - 1)
            rep_ctx.__exit__(None, None, None)
    nc.compile()
    return nc


def _get_nc():
    if "nc" not in _NC_CACHE:
        _NC_CACHE["nc"] = _build_attn()
    return _NC_CACHE["nc"]


def kernel(input_embeddings, token_attention_masks_source=None,
           token_attention_masks_target=None, encoder_output_embedding=None,
           w_query=None, w_key=None, w_value=None, **_unused):
    """Full inputs in, full output out. Runs on 8 NeuronCores (SPMD)."""
    input_embeddings = np.ascontiguousarray(
        np.asarray(input_embeddings, dtype=np.float32))
    w_query = np.asarray(w_query, dtype=np.float32)
    w_key = np.asarray(w_key, dtype=np.float32)
    w_value = np.asarray(w_value, dtype=np.float32)
    b_sz, s_src, d = input_embeddings.shape
    assert (b_sz, s_src, d) == (B_SZ, S_SRC, D), "kernel compiled for 2x4096x128"

    # the query-axis mask is a softmax no-op; masks/encoder inputs unused.
    shards_per_b = N_CORES // b_sz
    wq_s = (w_query / np.float32(np.sqrt(d))).astype(np.float32)
    in_maps = []
    for c in range(N_CORES):
        b, s = divmod(c, shards_per_b)
        x = np.ascontiguousarray(input_embeddings[b])
        xq = np.ascontiguousarray(x[s * N_ROWS:(s + 1) * N_ROWS])
        in_maps.append({"x": x, "xq": xq, "wq": wq_s,
                        "wk": np.ascontiguousarray(w_key),
                        "wv": np.ascontiguousarray(w_value)})

    res = run_bass_kernel_spmd(_get_nc(), in_maps, list(range(N_CORES)))
    out = np.empty((B_SZ, S_SRC, D), np.float32)
    for c in range(N_CORES):
        b, s = divmod(c, shards_per_b)
        out[b, s * N_ROWS:(s + 1) * N_ROWS] = res.results[c]["out"]
    return out
